# revision 4
# baseline (speedup 1.0000x reference)
"""RNN-T greedy decode on 8 Trainium2 NeuronCores (Bass/Tile).

Strategy (fully model-parallel, 2 tiny AllGathers per decode step):
 - Each core owns a 128-wide slice of each LSTM gate (Wx/Wh columns), a
   1/8 vocab shard of W_out, and a full copy of W_pred.
 - emb@Wx + b is precomputed on the host -> per-step embedding lookup is an
   indirect-DMA row gather (no Wx matmul on device).
 - enc@W_enc + b_joint is precomputed on the host per time step (transposed
   layout) -> no W_enc matmul on device.
 - Per decode step: committed-h LSTM shard -> AllGather h2 slices (4KB) ->
   joint + vocab-shard logits -> local argmax (max8/max_index) -> AllGather
   (value, index) candidates (64B) -> replicated global-argmax commit.
 - Inactive rows (t >= encoded_lens[b]) keep evolving on device; their
   h/c are snapshotted by static per-row DMAs at their last active step and
   labels are blank-filled on the host. This removes the active-mask from
   the device entirely (exact, since rows never interact).
"""
import numpy as np

B, T, D, H, J, V = 8, 200, 1024, 1024, 1024, 16384
NCORES = 8
BLANK = V            # 16384
VS = 2049            # per-core vocab shard (8*2049 = 16392 >= V+1)
HS = H // NCORES     # 128
U = 4                # max symbols per step


# ---------------------------------------------------------------- program ---
def _build_program(n_steps, snap_steps):
    from concourse import bass, bacc, mybir, tile
    from concourse.masks import make_identity

    f32 = mybir.dt.float32
    i32 = mybir.dt.int32
    u32 = mybir.dt.uint32
    AF = mybir.ActivationFunctionType
    OP = mybir.AluOpType

    max_len = n_steps // 4 + (1 if n_steps % 4 else 0)
    nc = bacc.Bacc("TRN2", target_bir_lowering=False, num_devices=NCORES)

    # ---- I/O ----
    embWx_d = nc.dram_tensor("embWx", [V + 1, 512], f32, kind="ExternalInput")
    Wh_d = nc.dram_tensor("Whs", [H, 512], f32, kind="ExternalInput")
    Wp_d = nc.dram_tensor("Wp", [H, J], f32, kind="ExternalInput")
    Wo_d = nc.dram_tensor("Wos", [J, VS], f32, kind="ExternalInput")
    bo_d = nc.dram_tensor("bo8", [8, VS], f32, kind="ExternalInput")
    bz_d = nc.dram_tensor("bz8", [8, 512], f32, kind="ExternalInput")
    cid_d = nc.dram_tensor("cid8", [8, 1], f32, kind="ExternalInput")
    encT_d = nc.dram_tensor("encT", [max_len, 128, 64], f32, kind="ExternalInput")

    labels_d = nc.dram_tensor("labels", [8, n_steps], i32, kind="ExternalOutput")
    h_out_d = nc.dram_tensor("h_out", [8, H], f32, kind="ExternalOutput")
    c_out_d = nc.dram_tensor("c_out", [8, HS], f32, kind="ExternalOutput")

    rg = [list(range(NCORES))]
    LCH = [(0, 512), (512, 512), (1024, 512), (1536, 512), (2048, 1)]

    with tile.TileContext(nc) as tc:
        with (
            tc.tile_pool(name="wpool", bufs=1) as wp,
            tc.tile_pool(name="state", bufs=1) as st,
            tc.tile_pool(name="work", bufs=3) as wk,
            tc.tile_pool(name="enc", bufs=2) as encp,
            tc.tile_pool(name="mail", bufs=2) as mailp,
            tc.tile_pool(name="zp", bufs=2, space="PSUM") as zp,
            tc.tile_pool(name="trp", bufs=2, space="PSUM") as trp,
            tc.tile_pool(name="jpp", bufs=2, space="PSUM") as jpp,
            tc.tile_pool(name="lpp", bufs=2, space="PSUM") as lpp,
            tc.tile_pool(name="dram", bufs=4, space="DRAM") as dr,
        ):
            # ---- prologue: weights + constants + state ----
            Wh_sb = wp.tile([128, 8 * 512], f32, name="Wh_sb")
            Wp_sb = wp.tile([128, 8 * 1024], f32, name="Wp_sb")
            Wo_sb = wp.tile([128, 8 * VS], f32, name="Wo_sb")
            bo_sb = wp.tile([8, VS], f32, name="bo_sb")
            cid_sb = wp.tile([8, 1], f32, name="cid_sb")
            ident = wp.tile([128, 128], f32, name="ident")
            ones1 = wp.tile([1, 128], f32, name="ones1")
            big = wp.tile([8, 8], f32, name="big")
            for k in range(8):
                nc.sync.dma_start(Wh_sb[:, 512 * k:512 * (k + 1)],
                                  Wh_d[128 * k:128 * (k + 1), :])
                nc.sync.dma_start(Wp_sb[:, 1024 * k:1024 * (k + 1)],
                                  Wp_d[128 * k:128 * (k + 1), :])
                nc.sync.dma_start(Wo_sb[:, VS * k:VS * (k + 1)],
                                  Wo_d[128 * k:128 * (k + 1), :])
            nc.sync.dma_start(bo_sb[:], bo_d[:])
            nc.sync.dma_start(cid_sb[:], cid_d[:])
            make_identity(nc, ident[:])
            nc.vector.memset(ones1[:], 1.0)
            nc.vector.memset(big[:], 1e9)

            hT = st.tile([128, 64], f32, name="hT")
            c_sb = st.tile([8, HS], f32, name="c_sb")
            c2_sb = st.tile([8, HS], f32, name="c2_sb")
            last_f = st.tile([8, 1], f32, name="last_f")
            lasti = st.tile([8, 1], i32, name="lasti")
            labels_sb = st.tile([8, n_steps], i32, name="labels_sb")
            nc.vector.memset(hT[:], 0.0)
            nc.vector.memset(c_sb[:], 0.0)
            nc.vector.memset(c2_sb[:], 0.0)
            nc.vector.memset(last_f[:], float(BLANK))
            nc.vector.tensor_copy(lasti[:], last_f[:])

            mail_prev = None     # h2T mailbox of step s-1
            ag2_prev = None      # candidate AG output of step s-1

            # ------------------------- winner / commit phase -------------
            def phase_w(s):
                """Consume AG2(s-1): global argmax, commits, snapshots, gather."""
                cu = wk.tile([8, 16], f32, name="cu")
                src = ag2_prev.rearrange("c b two -> b c two")
                nc.sync.dma_start(cu[:].rearrange("b (c two) -> b c two", two=2),
                                  src)
                vals = cu[:].rearrange("b (c two) -> b c two", two=2)[:, :, 0]
                gidx = cu[:].rearrange("b (c two) -> b c two", two=2)[:, :, 1]
                vmax = wk.tile([8, 1], f32, name="vmax")
                nc.vector.tensor_reduce(vmax[:], vals,
                                        axis=mybir.AxisListType.X, op=OP.max)
                eq = wk.tile([8, 8], u32, name="eq")
                nc.vector.tensor_tensor(eq[:], vals,
                                        vmax[:].to_broadcast([8, 8]),
                                        op=OP.is_ge)
                gm = wk.tile([8, 8], f32, name="gm")
                nc.vector.select(gm[:], eq[:], gidx, big[:])
                G = wk.tile([8, 1], f32, name="G")
                nc.vector.tensor_reduce(G[:], gm[:],
                                        axis=mybir.AxisListType.X, op=OP.min)
                nc.vector.tensor_copy(labels_sb[:, s - 1:s], G[:])
                advu = wk.tile([8, 1], u32, name="advu")
                advf = wk.tile([8, 1], f32, name="advf")
                nc.vector.tensor_scalar(advu[:], G[:], float(BLANK),
                                        scalar2=None, op0=OP.not_equal)
                nc.vector.tensor_scalar(advf[:], G[:], float(BLANK),
                                        scalar2=None, op0=OP.not_equal)
                # commits
                nc.vector.copy_predicated(last_f[:],
                                          advu[:].to_broadcast([8, 1]), G[:])
                nc.vector.tensor_copy(lasti[:], last_f[:])
                nc.vector.copy_predicated(c_sb[:],
                                          advu[:].to_broadcast([8, HS]),
                                          c2_sb[:])
                # replicate adv across partitions: ones1.T @ advrow
                advr_ps = trp.tile([1, 8], f32, name="advr_ps", tag="tr", space="PSUM")
                nc.tensor.transpose(advr_ps[:], advf[:], ident[:8, :8])
                advrow = wk.tile([1, 8], f32, name="advrow")
                nc.vector.tensor_copy(advrow[:], advr_ps[:])
                advrep = trp.tile([128, 8], f32, name="advrep", tag="tr", space="PSUM")
                nc.tensor.matmul(advrep[:], ones1[:1, :], advrow[:1, :],
                                 start=True, stop=True)
                advT = wk.tile([128, 8], u32, name="advT")
                nc.vector.tensor_copy(advT[:], advrep[:])
                for k in range(8):
                    nc.vector.copy_predicated(hT[:, 8 * k:8 * (k + 1)], advT[:],
                                              mail_prev[:, 8 * k:8 * (k + 1)])
                # snapshots (static per-row)
                for bb in range(8):
                    if snap_steps[bb] == s:
                        nc.sync.dma_start(
                            h_out_d.rearrange("bb (k p) -> bb p k", p=128)[bb],
                            hT[:].rearrange("p (k b) -> p k b", b=8)[:, :, bb])
                        nc.sync.dma_start(c_out_d[bb:bb + 1, :],
                                          c_sb[bb:bb + 1, :])
                # next-step embedding row gather
                xwx = wk.tile([8, 512], f32, name="xwx")
                nc.gpsimd.indirect_dma_start(
                    out=xwx[:], out_offset=None, in_=embWx_d[:],
                    in_offset=bass.IndirectOffsetOnAxis(ap=lasti[:, :1], axis=0))
                return xwx

            # ------------------------------- main loop -------------------
            encT_buf = None
            for s in range(n_steps):
                t = s // 4
                xwx = phase_w(s) if s > 0 else None

                # ---- LSTM shard ----
                z_sb = wk.tile([8, 512], f32, name="z_sb")
                if s == 0:
                    nc.sync.dma_start(z_sb[:], bz_d[:])
                else:
                    zps = zp.tile([8, 512], f32, name="zps", space="PSUM")
                    for k in range(8):
                        nc.tensor.matmul(zps[:], hT[:, 8 * k:8 * (k + 1)],
                                         Wh_sb[:, 512 * k:512 * (k + 1)],
                                         start=(k == 0), stop=(k == 7))
                    nc.vector.tensor_tensor(z_sb[:], zps[:], xwx[:], op=OP.add)
                # z layout is [i, f, o, g] (host-reordered gate slices)
                sg = wk.tile([8, 384], f32, name="sg")
                tg = wk.tile([8, HS], f32, name="tg")
                nc.scalar.activation(sg[:], z_sb[:, 0:384], AF.Sigmoid)
                nc.scalar.activation(tg[:], z_sb[:, 384:512], AF.Tanh)
                p1 = wk.tile([8, HS], f32, name="p1")
                nc.vector.tensor_tensor(p1[:], sg[:, 0:128], tg[:], op=OP.mult)
                p2 = wk.tile([8, HS], f32, name="p2")
                nc.vector.tensor_tensor(p2[:], sg[:, 128:256], c_sb[:],
                                        op=OP.mult)
                nc.vector.tensor_tensor(c2_sb[:], p1[:], p2[:], op=OP.add)
                tc2 = wk.tile([8, HS], f32, name="tc2")
                nc.scalar.activation(tc2[:], c2_sb[:], AF.Tanh)
                h2 = wk.tile([8, HS], f32, name="h2")
                nc.vector.tensor_tensor(h2[:], sg[:, 256:384], tc2[:],
                                        op=OP.mult)

                # ---- h2 slice AllGather -> h2T mailbox [128, 64] ----
                h2t_ps = trp.tile([128, 8], f32, name="h2t_ps", tag="tr", space="PSUM")
                nc.tensor.transpose(h2t_ps[:], h2[:], ident[:8, :8])
                h2T_self = wk.tile([128, 8], f32, name="h2T_self")
                nc.vector.tensor_copy(h2T_self[:], h2t_ps[:])
                ag1_in = dr.tile([128, 8], f32, name="ag1_in")
                ag1_out = dr.tile([8, 128, 8], f32, name="ag1_out",
                                  addr_space="Shared")
                nc.sync.dma_start(ag1_in[:], h2T_self[:])
                nc.gpsimd.collective_compute(
                    "AllGather", OP.bypass, replica_groups=rg,
                    ins=[ag1_in.opt()], outs=[ag1_out.opt()])
                mail = mailp.tile([128, 64], f32, name="mail")
                nc.sync.dma_start(
                    mail[:].rearrange("p (c b) -> p c b", b=8),
                    ag1_out.rearrange("c p b -> p c b"))

                # ---- joint ----
                if s % 4 == 0:
                    encT_buf = encp.tile([128, 64], f32, name="encT_buf")
                    nc.sync.dma_start(encT_buf[:], encT_d[t])
                jp_sb = wk.tile([8, 1024], f32, name="jp_sb")
                for m in range(2):
                    jps = jpp.tile([8, 512], f32, name="jps", space="PSUM")
                    for k in range(8):
                        nc.tensor.matmul(
                            jps[:], mail[:, 8 * k:8 * (k + 1)],
                            Wp_sb[:, 1024 * k + 512 * m:1024 * k + 512 * (m + 1)],
                            start=(k == 0), stop=(k == 7))
                    nc.vector.tensor_copy(jp_sb[:, 512 * m:512 * (m + 1)], jps[:])
                jpT = wk.tile([128, 64], f32, name="jpT")
                for k in range(8):
                    jtr = trp.tile([128, 8], f32, name="jtr", tag="tr", space="PSUM")
                    nc.tensor.transpose(jtr[:], jp_sb[:, 128 * k:128 * (k + 1)],
                                        ident[:8, :8])
                    nc.vector.tensor_copy(jpT[:, 8 * k:8 * (k + 1)], jtr[:])
                jointT = wk.tile([128, 64], f32, name="jointT")
                nc.vector.tensor_tensor(jointT[:], jpT[:], encT_buf[:], op=OP.add)
                nc.scalar.activation(jointT[:], jointT[:], AF.Tanh)

                # ---- vocab-shard logits + local argmax ----
                logits = wk.tile([8, VS], f32, name="logits")
                for off, w in LCH:
                    lps = lpp.tile([8, 512], f32, name="lps", space="PSUM")
                    for k in range(8):
                        nc.tensor.matmul(lps[:, :w],
                                         jointT[:, 8 * k:8 * (k + 1)],
                                         Wo_sb[:, VS * k + off:VS * k + off + w],
                                         start=(k == 0), stop=(k == 7))
                    nc.vector.tensor_tensor(logits[:, off:off + w], lps[:, :w],
                                            bo_sb[:, off:off + w], op=OP.add)
                mx = wk.tile([8, 8], f32, name="mx")
                mi = wk.tile([8, 8], u32, name="mi")
                nc.vector.max(out=mx[:], in_=logits[:])
                nc.vector.max_index(out=mi[:], in_max=mx[:], in_values=logits[:])
                gidxg = wk.tile([8, 1], f32, name="gidxg")
                nc.vector.tensor_copy(gidxg[:], mi[:, 0:1])
                nc.vector.tensor_tensor(gidxg[:], gidxg[:], cid_sb[:], op=OP.add)
                cand = wk.tile([8, 2], f32, name="cand")
                nc.vector.tensor_copy(cand[:, 0:1], mx[:, 0:1])
                nc.vector.tensor_copy(cand[:, 1:2], gidxg[:])
                ag2_in = dr.tile([8, 2], f32, name="ag2_in")
                ag2_out = dr.tile([8, 8, 2], f32, name="ag2_out",
                                  addr_space="Shared")
                nc.sync.dma_start(ag2_in[:], cand[:])
                nc.gpsimd.collective_compute(
                    "AllGather", OP.bypass, replica_groups=rg,
                    ins=[ag2_in.opt()], outs=[ag2_out.opt()])
                mail_prev = mail
                ag2_prev = ag2_out

            phase_w(n_steps)  # epilogue: final commit + snapshots + label
            nc.sync.dma_start(labels_d[:], labels_sb[:])
    nc.finalize()
    return nc


# ---------------------------------------------------------------- runner ----
class _Runner:
    def __init__(self, nc, n_cores):
        import jax
        from jax.sharding import Mesh, PartitionSpec
        from jax.experimental.shard_map import shard_map
        from concourse import mybir
        from concourse.bass2jax import (_bass_exec_p, partition_id_tensor,
                                        install_neuronx_cc_hook)
        install_neuronx_cc_hook()
        self.jax = jax
        self.n_cores = n_cores
        pname = nc.partition_id_tensor.name if nc.partition_id_tensor else None
        in_names, out_names, out_avals, zero_outs = [], [], [], []
        for alloc in nc.m.functions[0].allocations:
            if not isinstance(alloc, mybir.MemoryLocationSet):
                continue
            name = alloc.memorylocations[0].name
            if alloc.kind == "ExternalInput":
                if name != pname:
                    in_names.append(name)
            elif alloc.kind == "ExternalOutput":
                out_names.append(name)
                shape = tuple(alloc.tensor_shape)
                dtype = mybir.dt.np(alloc.dtype)
                out_avals.append(jax.core.ShapedArray(shape, dtype))
                zero_outs.append(np.zeros(shape, dtype))
        self.in_names, self.out_names = in_names, out_names
        n_params, n_outs = len(in_names), len(out_avals)
        all_in = in_names + out_names + ([pname] if pname else [])

        def _body(*args):
            operands = list(args)
            if pname is not None:
                operands.append(partition_id_tensor())
            outs = _bass_exec_p.bind(
                *operands, out_avals=tuple(out_avals), in_names=tuple(all_in),
                out_names=tuple(out_names), lowering_input_output_aliases=(),
                sim_require_finite=False, sim_require_nnan=False, nc=nc)
            return tuple(outs)

        devices = jax.devices()[:n_cores]
        mesh = Mesh(np.asarray(devices), ("core",))
        in_specs = (PartitionSpec("core"),) * (n_params + n_outs)
        out_specs = (PartitionSpec("core"),) * n_outs
        self._fn = jax.jit(
            shard_map(_body, mesh=mesh, in_specs=in_specs,
                      out_specs=out_specs, check_rep=False),
            keep_unused=True)
        self._zero_outs = zero_outs

    def prepare(self, in_maps):
        jax = self.jax
        per_core = [[np.asarray(m[n]) for n in self.in_names] for m in in_maps]
        concat = [np.concatenate([per_core[c][i] for c in range(self.n_cores)],
                                 axis=0) for i in range(len(self.in_names))]
        concat += [np.concatenate([z] * self.n_cores, axis=0)
                   for z in self._zero_outs]
        return [jax.device_put(a) for a in concat]

    def run_prepared(self, args):
        outs = self._fn(*args)
        self.jax.block_until_ready(outs)
        return outs

    def split(self, outs):
        res = [dict() for _ in range(self.n_cores)]
        for i, name in enumerate(self.out_names):
            for c, part in enumerate(np.split(np.asarray(outs[i]),
                                              self.n_cores, axis=0)):
                res[c][name] = part
        return res

    def run(self, in_maps):
        return self.split(self.run_prepared(self.prepare(in_maps)))


_CACHE = {}


def _get_runner(n_steps, snap_key):
    key = (n_steps, snap_key)
    if key not in _CACHE:
        nc = _build_program(n_steps, list(snap_key))
        _CACHE[key] = _Runner(nc, NCORES)
    return _CACHE[key]


# ---------------------------------------------------------------- kernel ----
def _prepare(encoded_outs, encoded_lens, emb, Wx, Wh, b, W_enc, W_pred,
             b_joint, W_out, b_out, n_steps_override=None):
    encoded_outs = np.asarray(encoded_outs, np.float32)
    lens = np.asarray(encoded_lens)
    emb = np.asarray(emb, np.float32); Wx = np.asarray(Wx, np.float32)
    Wh = np.asarray(Wh, np.float32); b = np.asarray(b, np.float32)
    W_enc = np.asarray(W_enc, np.float32)
    W_pred = np.asarray(W_pred, np.float32)
    b_joint = np.asarray(b_joint, np.float32)
    W_out = np.asarray(W_out, np.float32)
    b_out = np.asarray(b_out, np.float32)

    max_len = int(lens.max())
    n_steps = 4 * max_len if n_steps_override is None else n_steps_override
    enc_len = max(n_steps // 4 + (1 if n_steps % 4 else 0), 1)
    snap = tuple(int(min(4 * lens[bb], n_steps)) for bb in range(B))

    # ---- host precompute ----
    embWx = emb @ Wx + b                                     # [V+1, 4H]
    enc = encoded_outs.transpose(0, 2, 1)                    # [B, T, D]
    encterm = (enc[:, :enc_len].reshape(-1, D) @ W_enc).reshape(B, enc_len, J)
    encterm = (encterm + b_joint).astype(np.float32)
    # transposed layout: encT[t, p, 8k+b] = encterm[b, t, 128k+p]
    encT = np.ascontiguousarray(
        encterm.transpose(1, 2, 0).reshape(enc_len, 8, 128, 8)
        .transpose(0, 2, 1, 3).reshape(enc_len, 128, 64))

    in_maps = []
    for ccc in range(NCORES):
        gcols = np.concatenate([g * H + ccc * HS + np.arange(HS)
                                for g in (0, 1, 3, 2)])  # i, f, o, g
        lo = VS * ccc
        hi = min(lo + VS, V + 1)
        Wo_c = np.zeros((J, VS), np.float32)
        bo_c = np.full((VS,), -1e30, np.float32)
        Wo_c[:, :hi - lo] = W_out[:, lo:hi]
        bo_c[:hi - lo] = b_out[lo:hi]
        in_maps.append(dict(
            embWx=np.ascontiguousarray(embWx[:, gcols]),
            Whs=np.ascontiguousarray(Wh[:, gcols]),
            Wp=W_pred,
            Wos=Wo_c,
            bo8=np.tile(bo_c, (8, 1)),
            bz8=np.tile(b[gcols], (8, 1)),
            cid8=np.full((8, 1), float(VS * ccc), np.float32),
            encT=encT,
        ))

    return in_maps, n_steps, snap


def _assemble(res, lens, n_steps):
    labels_flat = res[0]["labels"]                  # [8, n_steps]
    labels = np.full((B, T, U), BLANK, np.int32)
    nfull = n_steps // 4
    labels[:, :nfull, :] = labels_flat[:, :nfull * 4].reshape(B, nfull, U)
    for bb in range(B):
        labels[bb, lens[bb]:, :] = BLANK
    h = res[0]["h_out"].astype(np.float32)
    c = np.zeros((B, H), np.float32)
    for ccc in range(NCORES):
        c[:, ccc * HS:(ccc + 1) * HS] = res[ccc]["c_out"]
    return labels, h, c


def kernel(encoded_outs, encoded_lens, emb, Wx, Wh, b, W_enc, W_pred,
           b_joint, W_out, b_out, n_steps_override=None):
    lens = np.asarray(encoded_lens)
    in_maps, n_steps, snap = _prepare(
        encoded_outs, encoded_lens, emb, Wx, Wh, b, W_enc, W_pred,
        b_joint, W_out, b_out, n_steps_override)
    runner = _get_runner(n_steps, snap)
    res = runner.run(in_maps)
    return _assemble(res, lens, n_steps)
